# revision 6
# baseline (speedup 1.0000x reference)
"""CoNystromAttention Trainium2 kernel, v3.

Shard: 8 cores = 4 batches x 2 head-groups (8 heads each). Per core:
one batch b, 8 heads organized as 4 "pairs" (2 heads = 128 partitions).

Math (reference, with Q=K=V=QKV):
  QKV = X[b].T @ Wq[h].T + bq[h]                       [n=4096, d=64]
  Qt  = window-mean(QKV, 64)                           [m=64, d]
  S   = exp(QKV @ Qt.T / 8)     (Beta; Delta = S.T)    [n, m]
  G   = exp(Qt @ Qt.T / 8); GD = G/rowsum(G)
  V6  = newton_schulz(GD, 6)  (pinv; per-head init scale)
  out = diag(1/r) S V6 diag(1/c) S.T QKV,  r=rowsum(S), c=colsum(S)

v3: bias added to QKV during the phase-1 PSUM->SBUF move (DVE, free);
qkvt kept f32r; S^T exp'd once (colsums via accumulator); token-major
S and QKV chunks produced per-chunk by PE transposes (QKV) and direct
matmuls + exp (S); r via ones-column matmuls in the final loop;
Newton-Schulz in bf16 128-wide, micro-stepped into the instruction
streams; no collective (per-head NS init scale).
"""

import numpy as np

P = 128
N_TOK = 4096
EMBED = 1024
NPAIR = 4            # head-pairs per core (8 heads)
ECH = EMBED // P     # 8 contraction chunks
XCH = 512            # projection chunk (tokens)
NCHP = N_TOK // XCH  # 8 projection chunks
NCH8 = N_TOK // 512  # 8 S^T chunks of 512
TCH = N_TOK // P     # 32 token chunks of 128
NS_ITERS = 6

_CACHE = {}


def _build():
    import concourse.mybir as mybir
    from concourse import bacc
    from concourse.tile import TileContext
    from concourse.masks import make_identity

    f32 = mybir.dt.float32
    f32r = mybir.dt.float32r
    bf16 = mybir.dt.bfloat16
    ALU = mybir.AluOpType
    ACTF = mybir.ActivationFunctionType
    AX = mybir.AxisListType

    nc = bacc.Bacc("TRN2", target_bir_lowering=False, debug=False)
    X = nc.dram_tensor("X", [EMBED, N_TOK], f32r, kind="ExternalInput")
    WqT = nc.dram_tensor("WqT", [EMBED, 512], f32r, kind="ExternalInput")
    bias = nc.dram_tensor("bias", [512], f32, kind="ExternalInput")
    out_d = nc.dram_tensor("out", [N_TOK, 512], bf16,
                           kind="ExternalOutput")

    with TileContext(nc) as tc, (
        tc.tile_pool(name="big", bufs=1)
    ) as big, tc.tile_pool(name="persist", bufs=1) as pers, tc.tile_pool(
        name="nsv", bufs=1
    ) as nsp, tc.tile_pool(name="wk", bufs=4) as wk:
        # ---------------- persistent tiles ----------------
        bias_t = pers.tile([P, NPAIR], f32, tag="bias")
        nc.sync.dma_start(bias_t[:], bias.rearrange("(f p) -> p f", p=P))
        ones1p = pers.tile([1, P], bf16, tag="ones1p")
        nc.vector.memset(ones1p[:], 1.0)
        idf = pers.tile([P, P], f32, tag="idf")
        make_identity(nc, idf[:])
        identr = pers.tile([P, P], f32r, tag="identr")
        nc.vector.tensor_copy(identr[:], idf[:])
        i7 = pers.tile([P, P], bf16, tag="i7")
        i15 = pers.tile([P, P], bf16, tag="i15")
        i13 = pers.tile([P, P], bf16, tag="i13")
        for t, v in ((i7, 7.0), (i15, 15.0), (i13, 13.0)):
            nc.vector.tensor_scalar_mul(t[:], idf[:], v)
        ones2 = pers.tile([P, 2], bf16, tag="ones2")
        nc.vector.memset(ones2[:], 0.0)
        nc.vector.memset(ones2[0:64, 0:1], 1.0)
        nc.vector.memset(ones2[64:128, 1:2], 1.0)
        ones128 = pers.tile([P, 1], bf16, tag="ones128")
        nc.vector.memset(ones128[:], 1.0)
        zsrc = pers.tile([P, 256], f32, tag="zsrc")
        nc.vector.memset(zsrc[:], 0.0)

        qsum = [pers.tile([P, 64], f32, tag=f"qsum{p}", name=f"qsum{p}")
                for p in range(NPAIR)]
        qkvt = big.tile([P, NPAIR, N_TOK], f32r, tag="qkvt")

        # ---------------- phase 1: projection (bias added in the move) ----
        with (
            tc.tile_pool(name="wq", bufs=1) as wqp,
            tc.tile_pool(name="x", bufs=2) as xpool,
            tc.tile_pool(name="pp", bufs=4, space="PSUM") as pp,
        ):
            wqtr = wqp.tile([P, ECH, 512], f32r, tag="wqtr")
            wre = WqT.rearrange("(eo p) hd -> p eo hd", p=P)
            nc.sync.dma_start(wqtr[:, 0:4, :], wre[:, 0:4, :])
            nc.sync.dma_start(wqtr[:, 4:8, :], wre[:, 4:8, :])
            xre = X.rearrange("(eo p) n -> p eo n", p=P)
            for c in range(NCHP):
                xt = xpool.tile([P, ECH, XCH], f32r, tag="xt")
                csl = slice(c * XCH, (c + 1) * XCH)
                nc.sync.dma_start(xt[:, 0:4, :], xre[:, 0:4, csl])
                nc.sync.dma_start(xt[:, 4:8, :], xre[:, 4:8, csl])
                for p in range(NPAIR):
                    ps = pp.tile([P, XCH], f32, tag="proj", name=f"ps{c}_{p}")
                    for e in range(ECH):
                        nc.tensor.matmul(
                            ps[:],
                            wqtr[:, e, p * P:(p + 1) * P],
                            xt[:, e, :],
                            start=(e == 0),
                            stop=(e == ECH - 1),
                        )
                    nc.vector.tensor_scalar(
                        qkvt[:, p, csl], ps[:], 1.0, bias_t[:, p:p + 1],
                        ALU.mult, ALU.add,
                    )
                    nc.vector.reduce_sum(
                        qsum[p][:, c * 8:(c + 1) * 8],
                        ps[:].rearrange("p (w t) -> p w t", t=64),
                        axis=AX.X,
                    )

        # ---------------- phase 2 ----------------
        with tc.tile_pool(name="big2", bufs=1) as big2:
            st = big2.tile([P, NPAIR, N_TOK], bf16, tag="st")
            cparts = pers.tile([P, NPAIR, NCH8], f32, tag="cparts")
            rvr_all = pers.tile([P, TCH, NPAIR, 2], f32, tag="rvr_all")

            with tc.tile_pool(name="nsps", bufs=3, space="PSUM") as nsps:
                # landmarks (blkq = qsum/64 + bias), block-diag per pair, f32r
                blkq = []
                blkq_pad = []
                for p in range(NPAIR):
                    bq_t = pers.tile([P, P], f32r, tag=f"blkq{p}",
                                     name=f"blkq{p}")
                    nc.vector.tensor_copy(bq_t[:], zsrc[:, 0:P])
                    nc.vector.tensor_scalar(
                        bq_t[0:64, 0:64], qsum[p][0:64, :], 1.0 / 64,
                        bias_t[0:64, p:p + 1], ALU.mult, ALU.add,
                    )
                    nc.vector.tensor_scalar(
                        bq_t[64:128, 64:128], qsum[p][64:128, :], 1.0 / 64,
                        bias_t[64:128, p:p + 1], ALU.mult, ALU.add,
                    )
                    blkq.append(bq_t)
                    bp = pers.tile([P, 256], f32r, tag=f"blkqp{p}",
                                   name=f"blkqp{p}")
                    nc.vector.tensor_copy(bp[:, P:], zsrc[:, 0:P])
                    nc.vector.tensor_copy(bp[:, 0:P], bq_t[:])
                    blkq_pad.append(bp)

                # Gamma -> GD -> per-head NS scale -> v0/vt0
                vstate = []
                for p in range(NPAIR):
                    psg = nsps.tile([P, P], f32, tag="nsb", name=f"psg{p}")
                    nc.tensor.matmul(psg[:], blkq[p][:], blkq[p][:],
                                     start=True, stop=True)
                    g = wk.tile([P, P], f32, tag="g", name=f"g{p}")
                    nc.scalar.activation(g[:], psg[:], ACTF.Exp, scale=0.125)
                    nc.vector.memset(g[0:64, 64:128], 0.0)
                    nc.vector.memset(g[64:128, 0:64], 0.0)
                    gs = wk.tile([P, 1], f32, tag="gs", name=f"gs{p}")
                    nc.vector.reduce_sum(gs[:], g[:], axis=AX.X)
                    gri = wk.tile([P, 1], f32, tag="gri", name=f"gri{p}")
                    nc.vector.reciprocal(gri[:], gs[:])
                    gd = nsp.tile([P, P], bf16, tag=f"gd{p}", name=f"gd{p}")
                    nc.vector.tensor_scalar_mul(gd[:], g[:], gri[:])
                    # K^T = GD^T = G @ diag(gri)  (G symmetric)
                    g_bf = wk.tile([P, P], bf16, tag="g_bf", name=f"g_bf{p}")
                    nc.scalar.copy(g_bf[:], g[:])
                    dmat = wk.tile([P, P], bf16, tag="dmat", name=f"dmat{p}")
                    nc.vector.tensor_scalar_mul(dmat[:], idf[:], gri[:])
                    pskt = nsps.tile([P, P], f32, tag="nsb", name=f"pskt{p}")
                    nc.tensor.matmul(pskt[:], g_bf[:], dmat[:],
                                     start=True, stop=True)
                    kt = nsp.tile([P, P], bf16, tag=f"kt{p}", name=f"kt{p}")
                    nc.scalar.copy(kt[:], pskt[:])
                    # per-head max colsum: ones^T @ gd -> [1, P]
                    csps = nsps.tile([1, P], f32, tag="nsb", name=f"csps{p}")
                    nc.tensor.matmul(csps[:], ones128[:], gd[:],
                                     start=True, stop=True)
                    cm = wk.tile([1, 2], f32, tag="cm", name=f"cm{p}")
                    nc.vector.reduce_max(
                        cm[:], csps[:].rearrange("o (h l) -> o h l", l=64),
                        axis=AX.X,
                    )
                    cmr = wk.tile([1, 2], bf16, tag="cmr", name=f"cmr{p}")
                    with nc.allow_low_precision(reason="NS init scale"):
                        nc.vector.reciprocal(cmr[:], cm[:])
                    bps = nsps.tile([P, 2], f32, tag="nsb", name=f"bps{p}")
                    nc.tensor.matmul(bps[:], ones1p[:], cmr[:],
                                     start=True, stop=True)
                    sv = wk.tile([P, 1], f32, tag="sv", name=f"sv{p}")
                    nc.vector.tensor_copy(sv[0:64, :], bps[0:64, 0:1])
                    nc.vector.tensor_copy(sv[64:128, :], bps[64:128, 1:2])
                    v0 = nsp.tile([P, P], bf16, tag=f"v{p}", name=f"v0_{p}")
                    nc.vector.tensor_scalar_mul(v0[:], kt[:], sv[:])
                    vt0 = nsp.tile([P, P], bf16, tag=f"vt{p}", name=f"vt0_{p}")
                    nc.vector.tensor_scalar_mul(vt0[:], gd[:], sv[:])
                    vstate.append([kt, v0, vt0])

                # ---- Newton-Schulz micro-step stream ----
                ns_live = {}

                def ns_micro(it, p, s):
                    kt, v, vt = vstate[p]
                    if s == 0:
                        pskv = nsps.tile([P, P], f32, tag="nsb",
                                         name=f"pskv{p}_{it}")
                        nc.tensor.matmul(pskv[:], kt[:], v[:],
                                         start=True, stop=True)
                        pskvt = nsps.tile([P, P], f32, tag="nsb",
                                          name=f"pskvt{p}_{it}")
                        nc.tensor.matmul(pskvt[:], v[:], kt[:],
                                         start=True, stop=True)
                        kvt = nsp.tile([P, P], bf16, tag=f"kvt{p}",
                                       name=f"kvt{p}_{it}")
                        nc.vector.tensor_copy(kvt[:], pskvt[:])
                        a1 = nsp.tile([P, P], bf16, tag=f"a1{p}",
                                      name=f"a1{p}_{it}")
                        nc.vector.tensor_tensor(a1[:], i7[:], pskv[:],
                                                ALU.subtract)
                        ns_live[p] = (kvt, a1)
                    elif s == 1:
                        kvt, a1 = ns_live[p]
                        psa2 = nsps.tile([P, P], f32, tag="nsb",
                                         name=f"psa2{p}_{it}")
                        nc.tensor.matmul(psa2[:], kvt[:], a1[:],
                                         start=True, stop=True)
                        a3 = nsp.tile([P, P], bf16, tag=f"a3{p}",
                                      name=f"a3{p}_{it}")
                        nc.vector.tensor_tensor(a3[:], i15[:], psa2[:],
                                                ALU.subtract)
                        ns_live[p] = (kvt, a3)
                    elif s == 2:
                        kvt, a3 = ns_live[p]
                        psa4 = nsps.tile([P, P], f32, tag="nsb",
                                         name=f"psa4{p}_{it}")
                        nc.tensor.matmul(psa4[:], kvt[:], a3[:],
                                         start=True, stop=True)
                        a5 = nsp.tile([P, P], bf16, tag=f"a5{p}",
                                      name=f"a5{p}_{it}")
                        nc.vector.tensor_tensor(a5[:], i13[:], psa4[:],
                                                ALU.subtract)
                        ns_live[p] = (a5,)
                    else:
                        (a5,) = ns_live[p]
                        if it < NS_ITERS - 1:
                            psv = nsps.tile([P, P], f32, tag="nsb",
                                            name=f"psv{p}_{it}")
                            nc.tensor.matmul(psv[:], vt[:], a5[:],
                                             start=True, stop=True)
                            vn = nsp.tile([P, P], bf16, tag=f"v{p}",
                                          name=f"vn{p}_{it}")
                            nc.vector.tensor_scalar_mul(vn[:], psv[:], 0.25)
                        else:
                            vn = v
                        psvt2 = nsps.tile([P, P], f32, tag="nsb",
                                          name=f"psvt2{p}_{it}")
                        nc.tensor.matmul(psvt2[:], a5[:], vt[:],
                                         start=True, stop=True)
                        vtn = nsp.tile([P, P], bf16, tag=f"vt{p}",
                                       name=f"vtn{p}_{it}")
                        nc.vector.tensor_scalar_mul(vtn[:], psvt2[:], 0.25)
                        vstate[p] = [kt, vn, vtn]

                ns_sched = [(it, p, s) for it in range(NS_ITERS)
                            for p in range(NPAIR) for s in range(4)]
                ns_i = [0]

                def ns_pump(k):
                    for _ in range(k):
                        if ns_i[0] < len(ns_sched):
                            ns_micro(*ns_sched[ns_i[0]])
                            ns_i[0] += 1

                # ---- merged S^T + M loop (independent until W) ----
                with (
                    tc.tile_pool(name="stps", bufs=1, space="PSUM") as stps,
                    tc.tile_pool(name="mps", bufs=1, space="PSUM") as mps,
                    tc.tile_pool(name="qps", bufs=1, space="PSUM") as qps,
                    tc.tile_pool(name="sps2", bufs=1, space="PSUM") as sps2,
                    tc.tile_pool(name="sps3", bufs=1, space="PSUM") as sps3,
                    tc.tile_pool(name="mv", bufs=3) as mvp,
                ):
                    mbank = mps.tile([P, NPAIR, P], f32, tag="mb", name="mbank")
                    for e in range(TCH):
                        p_st, ch = divmod(e, NCH8)
                        sl = slice(ch * 512, (ch + 1) * 512)
                        psst = stps.tile([P, 512], f32, tag="stp",
                                         name=f"st{p_st}_{ch}")
                        nc.tensor.matmul(
                            psst[:], blkq[p_st][:], qkvt[:, p_st, sl],
                            start=True, stop=True,
                        )
                        nc.scalar.activation(
                            st[:, p_st, sl], psst[:], ACTF.Exp,
                            scale=0.125,
                            accum_out=cparts[:, p_st, ch:ch + 1],
                        )
                        tsl = slice(e * P, (e + 1) * P)
                        psq = qps.tile([P, 512], f32r, tag="qT", name=f"psq{e}")
                        for p in range(NPAIR):
                            nc.tensor.matmul(
                                psq[:, p * P:(p + 1) * P], qkvt[:, p, tsl],
                                identr[:], is_transpose=True,
                                start=(p == 0), stop=(p == NPAIR - 1),
                                skip_group_check=True,
                            )
                        qnb = mvp.tile([P, 512], bf16, tag="qnb",
                                       name=f"qnb{e}")
                        nc.vector.tensor_copy(qnb[:], psq[:])
                        pssA = sps2.tile([P, 2, 256], f32, tag="snpA",
                                         name=f"pssA{e}")
                        pssB = sps3.tile([P, 2, 256], f32, tag="snpB",
                                         name=f"pssB{e}")
                        for p in range(NPAIR):
                            tgt = pssA if p < 2 else pssB
                            nc.tensor.matmul(
                                tgt[:, p % 2, :], qkvt[:, p, tsl],
                                blkq_pad[p][:],
                                start=(p % 2 == 0), stop=(p % 2 == 1),
                                skip_group_check=True,
                            )
                        snb = mvp.tile([P, NPAIR, P], bf16, tag="snb",
                                       name=f"snb{e}")
                        nc.scalar.activation(
                            snb[:, 0:2, :], pssA[:, :, 0:P], ACTF.Exp,
                            scale=0.125,
                        )
                        nc.scalar.activation(
                            snb[:, 2:4, :], pssB[:, :, 0:P], ACTF.Exp,
                            scale=0.125,
                        )
                        for p in range(NPAIR):
                            nc.tensor.matmul(
                                mbank[:, p, :], snb[:, p, :],
                                qnb[:, p * P:(p + 1) * P],
                                start=(e == 0 and p == 0),
                                stop=(e == TCH - 1 and p == NPAIR - 1),
                                skip_group_check=True,
                            )
                        if e >= 2:
                            ns_pump(3)
                        if e >= 24:
                            k24 = e - 24
                            for fc in range(4 * k24, 4 * k24 + 4):
                                fsl = slice(fc * P, (fc + 1) * P)
                                rp = nsps.tile([P, NPAIR, 2], f32, tag="nsb",
                                               name=f"rp{fc}")
                                for p in range(NPAIR):
                                    nc.tensor.matmul(
                                        rp[:, p, :], st[:, p, fsl], ones2[:],
                                        start=(p == 0), stop=(p == NPAIR - 1),
                                        skip_group_check=True,
                                    )
                                nc.vector.reciprocal(rvr_all[:, fc, :, :],
                                                     rp[:])
                    ns_pump(len(ns_sched))  # flush any leftovers

                    # ---- c, dvp, W ----
                    cs = wk.tile([P, NPAIR], f32, tag="cs")
                    nc.vector.reduce_sum(cs[:], cparts[:], axis=AX.X)
                    cinv = wk.tile([P, NPAIR], f32, tag="cinv")
                    nc.vector.reciprocal(cinv[:], cs[:])
                    dvp = pers.tile([P, NPAIR, P], bf16, tag="dvp")
                    nc.vector.tensor_tensor(
                        dvp[:], mbank[:],
                        cinv[:].rearrange("p (f o) -> p f o", o=1).to_broadcast(
                            [P, NPAIR, P]),
                        ALU.mult,
                    )
                    nc.vector.memset(dvp[0:64, :, 64:128], 0.0)
                    nc.vector.memset(dvp[64:128, :, 0:64], 0.0)
                    wps = mps.tile([P, NPAIR, P], f32, tag="mb", name="wps")
                    for p in range(NPAIR):
                        nc.tensor.matmul(
                            wps[:, p, :], vstate[p][2][:], dvp[:, p, :],
                            start=(p == 0), stop=(p == NPAIR - 1),
                            skip_group_check=True,
                        )
                    wpad = pers.tile([P, NPAIR, P], bf16, tag="wpad")
                    nc.scalar.copy(wpad[:], wps[:])

            # ---- final: out = diag(1/r) S W ----
            with (
                tc.tile_pool(name="fps", bufs=3, space="PSUM") as fps,
            ):
                for c in range(TCH):
                    tsl = slice(c * P, (c + 1) * P)
                    pso = fps.tile([P, NPAIR, P], f32, tag="fin",
                                   name=f"pso{c}")
                    for p in range(NPAIR):
                        nc.tensor.matmul(
                            pso[:, p, :], st[:, p, tsl], wpad[:, p, :],
                            start=(p == 0), stop=(p == NPAIR - 1),
                            skip_group_check=True,
                        )
                    ot = wk.tile([P, 512], bf16, tag="ot", name=f"ot{c}",
                                 bufs=8)
                    nc.vector.tensor_tensor(
                        ot[:].rearrange("p (f h d) -> p f h d", h=2, d=64),
                        pso[:].rearrange("p f (h d) -> p f h d", d=64),
                        rvr_all[:, c, :, :].rearrange(
                            "p f (h o) -> p f h o", o=1)
                            .to_broadcast([P, NPAIR, 2, 64]),
                        ALU.mult,
                    )
                    nc.sync.dma_start(out_d[tsl, :], ot[:])

    nc.compile()
    return nc


def _get_nc():
    if "nc" not in _CACHE:
        _CACHE["nc"] = _build()
    return _CACHE["nc"]


def kernel(X, Wq, bq):
    from concourse.bass_utils import run_bass_kernel_spmd

    nc = _get_nc()
    B, E, n = X.shape
    H = Wq.shape[0]
    in_maps = []
    for core in range(8):
        b = core // 2
        h0 = 8 * (core % 2)
        wq_c = Wq[h0:h0 + 8]                      # [8, 64, 1024]
        wqt_c = np.ascontiguousarray(wq_c.transpose(2, 0, 1).reshape(E, 512))
        bias_c = np.ascontiguousarray(bq[h0:h0 + 8].reshape(512))
        in_maps.append({
            "X": np.ascontiguousarray(X[b]),
            "WqT": wqt_c,
            "bias": bias_c,
        })
    res = run_bass_kernel_spmd(nc, in_maps, core_ids=list(range(8)))
    out = np.empty((B, H, n, 64), dtype=np.float32)
    for core in range(8):
        b = core // 2
        h0 = 8 * (core % 2)
        oc = np.asarray(res.results[core]["out"], dtype=np.float32)
        out[b, h0:h0 + 8] = oc.reshape(n, 8, 64).transpose(1, 0, 2)
    return out
